# revision 14
# baseline (speedup 1.0000x reference)
"""Causal multi-head attention (B=2, S=2048, D=1024, H=16) on 8 TRN2 NeuronCores.

Sharding: core c handles batch b = c//4 and heads [4*(c%4), 4*(c%4)+4).
Each core computes its 4 heads' attention plus the partial w_o projection;
the host sums the 4 partials per batch (the "all-reduce after w_o") and
adds the output bias plus the v-bias term folded through the softmax
(sum_k p_k (v+bv) / den = av/den + bv, and bv^T w_o is a constant row).

Compute dtype: bf16 matmul inputs with fp32 PSUM accumulation; y partials
written back in bf16 (summed in fp32 on host).

Layouts (per core, host-prepared):
  xT    [1024, 2048] bf16  x[b].T                    (d on partitions)
  wqk   [1024, 512]  bf16  cols = [k_h0..k_h3 | q_h0..q_h3] (64 each)
  wv    [1024, 256]  bf16  cols = [v_h0..v_h3]
  wo    [256, 1024]  bf16  w_o[:, head_cols].T
  bqk   [512, 1]     f32   per-feature bias, same col order as wqk
  masks [4, 128, 1024] bf16  causal 0/1 patterns for diagonal blocks

In-kernel dataflow (per core):
  qkvT = wqk.T @ x.T  -> k/q in [feature, seq] layout, paired heads per tile
  v    = x @ wv       -> natural [seq, feature] + a ones column per head
  scores_T[k,q] = k_h.T(dk x 128) @ q_h(dk x 512)   (two heads row-tiled)
  p = exp(scores/8)   (ACT, psum->sbuf, bf16) ; diag blocks masked via DVE
  av_T[dk+1, q] = [v_h | 1].T @ p                   (row 64 = softmax denom)
  avn = av * (1/denom broadcast)                    (DVE mul, -> bf16)
  y[s, o] = avn.T @ wo                              (partial, bf16 to HBM)

PSUM budget (8 banks): one shared 3-deep pool of [128,1024] tiles (6 banks)
serves scores, kq/v projection and w_o psum; av accumulators 2x[128,512].
Score-buffer depth 3 decouples the scores->exp->scores recycle so the PE
streams while ACT exps; w_o jobs are deferred to the last (largest)
attention tile where no QKV rounds remain to fill PE gaps.
"""

import numpy as np
import ml_dtypes

import concourse.bass as bass
import concourse.mybir as mybir
import concourse.tile as tile
from concourse.bass_utils import run_bass_kernel_spmd
from concourse.vector_clock import ScopedClock

BF16 = mybir.dt.bfloat16
F32 = mybir.dt.float32
NP_BF16 = ml_dtypes.bfloat16

B, S, D = 2, 2048, 1024
H, DK = 16, 64
HPC = 4            # heads per core
N_CORES = 8
S_TILE = 512       # q tile width (f32 psum bank)
K_BLK = 128        # k block (partition dim of transposed scores)


# ---------------------------------------------------------------------------
# Workaround: this walrus build rejects >1 sem-wait on the TileContext exit
# Drain. Redistribute the global-clock waits onto single-wait sync NOPs.
# ---------------------------------------------------------------------------
def _patched_drain_and_barrier(self, tick_clock, wait_clock):
    probe = self.nc.sync.nop()
    wait_clock.add_sem_waits(probe.ins, ScopedClock({None: tick_clock.global_clock}))
    si = probe.ins.sync_info
    waits = list(si.on_wait)
    probe.ins.sync_info = mybir.SyncInfo(on_wait=waits[:1], on_update=list(si.on_update))
    for w in waits[1:]:
        nop = self.nc.sync.nop()
        nop.ins.sync_info = mybir.SyncInfo(on_wait=[w], on_update=[])
    self.nc.sync.drain()

    self.nc.all_engine_barrier()
    assert self.sems is not None
    popped = self.nc._tile_sem_poison_stack.pop()
    assert popped is self._sem_poison
    self.nc.clear_and_free_semaphores(list(self.sems.allocated().values()))
    self.nc.all_engine_barrier()


tile.TileContext._drain_and_barrier = _patched_drain_and_barrier

_CFG_SALT = "cfg-v2"

_WAIT_LIMIT = 1


def _split_excess_waits(nc: bass.Bass, limit: int = _WAIT_LIMIT):
    """Walrus (this build) rejects instructions carrying more than a couple of
    sem waits. Move excess waits onto same-engine NOPs inserted just before."""
    n_split = 0
    for f in nc.m.functions:
        for bb in f.blocks:
            il = bb.instructions
            idx = 0
            while idx < len(il):
                inst = il[idx]
                si = inst.sync_info
                if si is not None and len(si.on_wait) > limit:
                    waits = list(si.on_wait)
                    pos = idx
                    for i in range(limit, len(waits), limit):
                        nop = mybir.InstNoOp(
                            name=f"{inst.name}_xw{i}", ins=[], outs=[]
                        )
                        nop.engine = inst.engine
                        nop.sync_info = mybir.SyncInfo(
                            on_wait=waits[i:i + limit], on_update=[]
                        )
                        il.insert(pos, nop)
                        pos += 1
                        idx += 1
                    inst.sync_info = mybir.SyncInfo(
                        on_wait=waits[:limit], on_update=list(si.on_update)
                    )
                    n_split += 1
                idx += 1
    return n_split


def build_attention_nc() -> bass.Bass:
    nc = bass.Bass("TRN2", target_bir_lowering=False, debug=False)

    xT_d = nc.dram_tensor("xT", [D, S], BF16, kind="ExternalInput").ap()
    wqk_d = nc.dram_tensor("wqk", [D, 8 * DK], BF16, kind="ExternalInput").ap()
    wv_d = nc.dram_tensor("wv", [D, 4 * DK], BF16, kind="ExternalInput").ap()
    wo_d = nc.dram_tensor("wo", [4 * DK, D], BF16, kind="ExternalInput").ap()
    bqk_d = nc.dram_tensor("bqk", [8 * DK, 1], F32, kind="ExternalInput").ap()
    masks_d = nc.dram_tensor("masks", [4, 128, 1024], BF16, kind="ExternalInput").ap()
    y_d = nc.dram_tensor("y", [S, D], BF16, kind="ExternalOutput").ap()

    n_kt = D // 128          # 8 contraction tiles over d
    n_st = S // 128          # 16 seq tiles of 128
    n_qt = S // S_TILE       # 4 q tiles of 512
    AV_LAG = 2               # AV trails exp by this many blocks in the stream

    from contextlib import ExitStack

    with tile.TileContext(nc) as tc, ExitStack() as stack:
        const = stack.enter_context(tc.tile_pool(name="const", bufs=1))
        xpool = stack.enter_context(tc.tile_pool(name="xp", bufs=1))
        kqpool = stack.enter_context(tc.tile_pool(name="kqp", bufs=1))
        vpool = stack.enter_context(tc.tile_pool(name="vp", bufs=1))
        avnpool = stack.enter_context(tc.tile_pool(name="avnp", bufs=1))
        ppool = stack.enter_context(tc.tile_pool(name="pp", bufs=24))
        spool = stack.enter_context(tc.tile_pool(name="sp", bufs=4))
        ypool = stack.enter_context(tc.tile_pool(name="yp", bufs=2))
        avsb = stack.enter_context(tc.tile_pool(name="avsb", bufs=4))
        rec_dr = stack.enter_context(tc.tile_pool(name="rec_dr", bufs=4, space="DRAM"))
        # PSUM: shared 3-deep [128,1024] pool (6 banks) + av pair (2 banks)
        big_ps = stack.enter_context(tc.tile_pool(name="big_ps", bufs=3, space="PSUM"))
        av_ps = stack.enter_context(tc.tile_pool(name="av_ps", bufs=2, space="PSUM"))

        # --- resident loads (ordered so QKV compute can start early) ----
        xT, wqk, wv = [], [], []
        for i in range(n_kt):
            t = xpool.tile([128, S], BF16, tag=f"xT{i}", name=f"xT{i}")
            xT.append(t)
        for i in range(n_kt):
            w1 = const.tile([128, 8 * DK], BF16, tag=f"wqk{i}", name=f"wqk{i}")
            nc.scalar.dma_start(out=w1, in_=wqk_d[i * 128:(i + 1) * 128, :])
            wqk.append(w1)
            w2 = const.tile([128, 4 * DK], BF16, tag=f"wv{i}", name=f"wv{i}")
            nc.scalar.dma_start(out=w2, in_=wv_d[i * 128:(i + 1) * 128, :])
            wv.append(w2)
            nc.sync.dma_start(
                out=xT[i][:, 0:S_TILE], in_=xT_d[i * 128:(i + 1) * 128, 0:S_TILE]
            )
        for sq in range(1, n_qt):
            for i in range(n_kt):
                nc.sync.dma_start(
                    out=xT[i][:, sq * S_TILE:(sq + 1) * S_TILE],
                    in_=xT_d[i * 128:(i + 1) * 128, sq * S_TILE:(sq + 1) * S_TILE],
                )
        bqk = []
        for i in range(4):
            t = const.tile([128, 1], F32, tag=f"bqk{i}", name=f"bqk{i}")
            nc.scalar.dma_start(out=t, in_=bqk_d[i * 128:(i + 1) * 128, :])
            bqk.append(t)
        masks = []
        for i in range(4):
            t = const.tile([128, 1024], BF16, tag=f"mask{i}", name=f"mask{i}")
            nc.scalar.dma_start(out=t, in_=masks_d[i])
            masks.append(t)
        wo = []
        for i in range(2):
            t = const.tile([128, D], BF16, tag=f"wo{i}", name=f"wo{i}")
            nc.scalar.dma_start(out=t, in_=wo_d[i * 128:(i + 1) * 128, :])
            wo.append(t)

        # kq[m][f, s]: m=0 -> k heads(0,1); 1 -> k heads(2,3); 2 -> q(0,1); 3 -> q(2,3)
        kq = [kqpool.tile([128, S], BF16, tag=f"kq{m}", name=f"kq{m}") for m in range(4)]
        # v_sb[st][128, 4*65]: per head h: cols [h*65, h*65+64) = v, col h*65+64 = 1.0
        v_sb = [vpool.tile([128, HPC * (DK + 1)], BF16, tag=f"v{st}", name=f"v{st}")
                for st in range(n_st)]
        # avn[f2][f, s]: f2=0 -> heads (0,1); f2=1 -> heads (2,3)
        avn = [avnpool.tile([128, S], BF16, tag=f"avn{f2}", name=f"avn{f2}")
               for f2 in range(2)]

        def emit_kq(m, sq):
            ps = big_ps.tile([128, S_TILE], F32, tag="bigps", name="qkps")
            for kt in range(n_kt):
                nc.tensor.matmul(
                    ps,
                    lhsT=wqk[kt][:, m * 128:(m + 1) * 128],
                    rhs=xT[kt][:, sq * S_TILE:(sq + 1) * S_TILE],
                    start=(kt == 0),
                    stop=(kt == n_kt - 1),
                )
            # psum -> sbuf with per-feature bias, on ACT (idle in QKV phase)
            nc.scalar.activation(
                kq[m][:, sq * S_TILE:(sq + 1) * S_TILE], ps,
                mybir.ActivationFunctionType.Identity, bias=bqk[m],
            )

        def emit_v(st):
            ps = big_ps.tile([128, HPC * DK], F32, tag="bigps", name="vps")
            for kt in range(n_kt):
                nc.tensor.matmul(
                    ps,
                    lhsT=xT[kt][:, st * 128:(st + 1) * 128],
                    rhs=wv[kt],
                    start=(kt == 0),
                    stop=(kt == n_kt - 1),
                )
            nc.gpsimd.memset(v_sb[st], 1.0)
            for h in range(HPC):
                nc.vector.tensor_copy(
                    out=v_sb[st][:, h * (DK + 1):h * (DK + 1) + DK],
                    in_=ps[:, h * DK:(h + 1) * DK],
                )

        def emit_wo(st):
            yp = big_ps.tile([128, D], F32, tag="bigps", name="yps")
            for oh in range(2):
                for f2 in range(2):
                    nc.tensor.matmul(
                        yp[:, oh * 512:(oh + 1) * 512],
                        lhsT=avn[f2][:, st * 128:(st + 1) * 128],
                        rhs=wo[f2][:, oh * 512:(oh + 1) * 512],
                        start=(f2 == 0),
                        stop=(f2 == 1),
                    )
            y_sb = ypool.tile([128, D], BF16, tag="ysb", name="ysb")
            nc.vector.tensor_copy(out=y_sb, in_=yp)
            nc.sync.dma_start(out=y_d[st * 128:(st + 1) * 128, :], in_=y_sb)

        def qkv_round(sq):
            return [
                lambda m=m, sq=sq: emit_kq(m, sq) for m in (0, 2, 1, 3)
            ] + [lambda st=st: emit_v(st) for st in range(4 * sq, 4 * sq + 4)]

        def attention_tile(t, jobs):
            """Emit attention for q-tile t, interleaving `jobs` (QKV groups of
            the next round, deferred w_o tiles) into the stream. AV matmuls
            trail their exp by AV_LAG blocks so the in-order PE stream never
            parks on an unfinished exp."""
            nblk = 4 * t + 4
            stride = max(1, (2 * nblk) // max(1, len(jobs)))
            s = 0
            for hp in range(2):
                kt2 = kq[hp]
                qt2 = kq[2 + hp]
                av_t = [av_ps.tile([128, S_TILE], F32, tag="avps", name="avps")
                        for _ in range(2)]
                pend = []

                def emit_av(blk, p):
                    for i in range(2):
                        h = 2 * hp + i
                        nc.tensor.matmul(
                            av_t[i][0:DK + 1, :],
                            lhsT=v_sb[blk][:, h * (DK + 1):(h + 1) * (DK + 1)],
                            rhs=p[:, i * S_TILE:(i + 1) * S_TILE],
                            start=(blk == 0),
                            stop=(blk == nblk - 1),
                        )

                for blk in range(nblk):
                    if jobs and s % stride == 0:
                        jobs.pop(0)()
                    s += 1
                    sc = big_ps.tile([128, 2 * S_TILE], F32, tag="bigps", name="scps")
                    for i in range(2):  # head A / head B, row-tiled pair
                        nc.tensor.matmul(
                            sc[:, i * S_TILE:(i + 1) * S_TILE],
                            lhsT=kt2[i * 64:(i + 1) * 64, blk * K_BLK:(blk + 1) * K_BLK],
                            rhs=qt2[i * 64:(i + 1) * 64, t * S_TILE:(t + 1) * S_TILE],
                            start=True,
                            stop=True,
                            tile_position=(i * 64, 0),
                        )
                    p = ppool.tile([128, 2 * S_TILE], BF16, tag="p", name="p")
                    nc.scalar.activation(p, sc, mybir.ActivationFunctionType.Exp,
                                         scale=0.125)
                    dd = blk - 4 * t
                    if dd >= 0:       # diagonal block: apply causal 0/1 mask
                        nc.vector.tensor_mul(p, p, masks[dd])
                    pend.append((blk, p))
                    if len(pend) > AV_LAG:
                        emit_av(*pend.pop(0))
                while pend:
                    if jobs and s % stride == 0:
                        jobs.pop(0)()
                    s += 1
                    emit_av(*pend.pop(0))
                # move av (+denominator row) off PSUM right away
                av_c = []
                for i in range(2):
                    c = avsb.tile([DK + 1, S_TILE], F32, tag="avc", name="avc")
                    nc.vector.tensor_copy(out=c, in_=av_t[i][0:DK + 1, :])
                    av_c.append(c)
                # normalize: in-place fast reciprocal of the denominator row,
                # GPSIMD partition-broadcast down to the 64 dk rows, multiply.
                for i in range(2):
                    den = av_c[i][DK:DK + 1, :]
                    nc.vector.reciprocal(out=den, in_=den)
                    rec_d = rec_dr.tile([S_TILE], F32, tag="recd", name="recd")
                    nc.gpsimd.dma_start(out=rec_d.rearrange("(a f) -> a f", a=1),
                                        in_=den)
                    bc = spool.tile([DK, S_TILE], F32, tag="bc", name="bc")
                    nc.gpsimd.dma_start(
                        out=bc,
                        in_=rec_d.rearrange("(a f) -> a f", a=1)
                                 .partition_broadcast(DK),
                    )
                    if i == 0:
                        dst = avn[hp][0:DK, t * S_TILE:(t + 1) * S_TILE]
                        nc.vector.tensor_mul(dst, av_c[i][0:DK, :], bc)
                    else:
                        tmp = spool.tile([DK, S_TILE], BF16, tag="avtmp", name="avtmp")
                        nc.vector.tensor_mul(tmp, av_c[i][0:DK, :], bc)
                        nc.gpsimd.dma_start(
                            out=avn[hp][64:128, t * S_TILE:(t + 1) * S_TILE],
                            in_=tmp,
                        )
            while jobs:
                jobs.pop(0)()

        for job in qkv_round(0):
            job()
        for t in range(n_qt):
            if t + 1 < n_qt:
                jobs = list(qkv_round(t + 1))
            else:
                # last tile: no QKV rounds left — fill PE gaps with the
                # deferred w_o projections of tiles 0..2
                jobs = [lambda st=st: emit_wo(st) for st in range(12)]
            attention_tile(t, jobs)
        for st in range(12, n_st):
            emit_wo(st)

    n = _split_excess_waits(nc)
    salt = mybir.InstNoOp(name=f"salt_{_CFG_SALT}", ins=[], outs=[])
    salt.engine = mybir.EngineType.SP
    nc.m.functions[0].blocks[0].instructions.insert(0, salt)
    return nc


_CACHED_NC = None


def _get_nc():
    global _CACHED_NC
    if _CACHED_NC is None:
        _CACHED_NC = build_attention_nc()
    return _CACHED_NC


def _prep_core_inputs(x, mask, w_qkv_w, w_qkv_b, w_o_w, w_o_b, core):
    b = core // 4
    hg = core % 4
    heads = [hg * HPC + h for h in range(HPC)]

    xT = np.ascontiguousarray(x[b].T).astype(NP_BF16)

    def rows(sec, h):  # q=0, k=1, v=2
        base = sec * D + h * DK
        return slice(base, base + DK)

    wqk_rows = np.concatenate(
        [w_qkv_w[rows(1, h)] for h in heads] + [w_qkv_w[rows(0, h)] for h in heads],
        axis=0,
    )  # [512, 1024]
    wqk = np.ascontiguousarray(wqk_rows.T).astype(NP_BF16)

    wv_rows = np.concatenate([w_qkv_w[rows(2, h)] for h in heads], axis=0)
    wv = np.ascontiguousarray(wv_rows.T).astype(NP_BF16)

    wo = np.ascontiguousarray(
        w_o_w[:, hg * HPC * DK:(hg + 1) * HPC * DK].T
    ).astype(NP_BF16)

    bqk = np.concatenate(
        [w_qkv_b[rows(1, h)] for h in heads] + [w_qkv_b[rows(0, h)] for h in heads]
    ).astype(np.float32)[:, None]

    # Diagonal-block mask patterns from the provided mask tensor.
    m2d = np.asarray(mask[0, 0])
    q0 = S - S_TILE
    pats = []
    for dd in range(4):
        k0 = q0 + dd * K_BLK
        pat = m2d[q0:q0 + S_TILE, k0:k0 + K_BLK].T.astype(np.float32)  # [128, 512]
        pats.append(np.concatenate([pat, pat], axis=1))               # [128, 1024]
    masks_np = np.stack(pats).astype(NP_BF16)

    return {
        "xT": xT, "wqk": wqk, "wv": wv, "wo": wo,
        "bqk": bqk, "masks": masks_np,
    }


def kernel(x, mask, w_qkv_w, w_qkv_b, w_o_w, w_o_b, _profile=False):
    x = np.asarray(x, np.float32)
    w_qkv_w = np.asarray(w_qkv_w, np.float32)
    w_qkv_b = np.asarray(w_qkv_b, np.float32)
    w_o_w = np.asarray(w_o_w, np.float32)
    w_o_b = np.asarray(w_o_b, np.float32)

    nc = _get_nc()
    in_maps = [
        _prep_core_inputs(x, mask, w_qkv_w, w_qkv_b, w_o_w, w_o_b, c)
        for c in range(N_CORES)
    ]
    res = run_bass_kernel_spmd(
        nc, in_maps, core_ids=list(range(N_CORES)), trace=_profile
    )
    y = np.zeros((B, S, D), np.float32)
    for c in range(N_CORES):
        y[c // 4] += np.asarray(res.results[c]["y"]).astype(np.float32)
    # bias: w_o bias plus the v-bias folded through the softmax average
    y += (w_o_b + w_o_w @ w_qkv_b[2 * D:3 * D])[None, None, :]
    if _profile:
        return y, res
    return y


# revision 18
# speedup vs baseline: 1.1854x; 1.1854x over previous
"""Causal multi-head attention (B=2, S=2048, D=1024, H=16) on 8 TRN2 NeuronCores.

Sharding: core c handles batch b = c//4 and heads [4*(c%4), 4*(c%4)+4).
Each core computes its 4 heads' attention plus the partial w_o projection;
the host sums the 4 partials per batch (the "all-reduce after w_o") and
adds the output bias plus the v-bias term folded through the softmax
(sum_k p_k (v+bv) / den = av/den + bv, and bv^T w_o is a constant row).

Compute dtype: bf16 matmul inputs with fp32 PSUM accumulation; y partials
written back in bf16 (summed in fp32 on host).

Layouts (per core, host-prepared):
  xT    [1024, 2048] bf16  x[b].T                    (d on partitions)
  wqk   [1024, 512]  bf16  cols = [k_h0..k_h3 | q_h0..q_h3] (64 each)
  wv    [1024, 256]  bf16  cols = [v_h0..v_h3]
  wo    [256, 1024]  bf16  w_o[:, head_cols].T
  bqk   [512, 1]     f32   per-feature bias, same col order as wqk
  masks [4, 128, 1024] bf16  causal 0/1 patterns for diagonal blocks

In-kernel dataflow (per core):
  qkvT = wqk.T @ x.T  -> k/q in [feature, seq] layout, paired heads per tile
  v    = x @ wv       -> natural [seq, feature] + a ones column per head
  scores_T[k,q] = k_h.T(dk x 128) @ q_h(dk x 512)   (two heads row-tiled)
  p = exp(scores/8)   (ACT, psum->sbuf, bf16) ; diag blocks masked via DVE
  av_T[dk+1, q] = [v_h | 1].T @ p                   (row 64 = softmax denom)
  avn = av * (1/denom broadcast)                    (DVE mul, -> bf16)
  y[s, o] = avn.T @ wo                              (partial, bf16 to HBM)

PSUM budget (8 banks): one shared 3-deep pool of [128,1024] tiles (6 banks)
serves scores, kq/v projection and w_o psum; av accumulators 2x[128,512].
Score-buffer depth 3 decouples the scores->exp->scores recycle so the PE
streams while ACT exps; w_o jobs are deferred to the last (largest)
attention tile where no QKV rounds remain to fill PE gaps.
"""

import numpy as np
import ml_dtypes

import concourse.bass as bass
import concourse.mybir as mybir
import concourse.tile as tile
from concourse.bass_utils import run_bass_kernel_spmd
from concourse.vector_clock import ScopedClock

BF16 = mybir.dt.bfloat16
F32 = mybir.dt.float32
NP_BF16 = ml_dtypes.bfloat16

B, S, D = 2, 2048, 1024
H, DK = 16, 64
HPC = 4            # heads per core
N_CORES = 8
S_TILE = 512       # q tile width (f32 psum bank)
K_BLK = 128        # k block (partition dim of transposed scores)


# ---------------------------------------------------------------------------
# Workaround: this walrus build rejects >1 sem-wait on the TileContext exit
# Drain. Redistribute the global-clock waits onto single-wait sync NOPs.
# ---------------------------------------------------------------------------
def _patched_drain_and_barrier(self, tick_clock, wait_clock):
    probe = self.nc.sync.nop()
    wait_clock.add_sem_waits(probe.ins, ScopedClock({None: tick_clock.global_clock}))
    si = probe.ins.sync_info
    waits = list(si.on_wait)
    probe.ins.sync_info = mybir.SyncInfo(on_wait=waits[:1], on_update=list(si.on_update))
    for w in waits[1:]:
        nop = self.nc.sync.nop()
        nop.ins.sync_info = mybir.SyncInfo(on_wait=[w], on_update=[])
    self.nc.sync.drain()

    self.nc.all_engine_barrier()
    assert self.sems is not None
    popped = self.nc._tile_sem_poison_stack.pop()
    assert popped is self._sem_poison
    self.nc.clear_and_free_semaphores(list(self.sems.allocated().values()))
    self.nc.all_engine_barrier()


tile.TileContext._drain_and_barrier = _patched_drain_and_barrier

_CFG_SALT = "cfg-v2"

_WAIT_LIMIT = 1


def _split_excess_waits(nc: bass.Bass, limit: int = _WAIT_LIMIT):
    """Walrus (this build) rejects instructions carrying more than a couple of
    sem waits. Move excess waits onto same-engine NOPs inserted just before."""
    n_split = 0
    for f in nc.m.functions:
        for bb in f.blocks:
            il = bb.instructions
            idx = 0
            while idx < len(il):
                inst = il[idx]
                si = inst.sync_info
                if si is not None and len(si.on_wait) > limit:
                    waits = list(si.on_wait)
                    pos = idx
                    for i in range(limit, len(waits), limit):
                        nop = mybir.InstNoOp(
                            name=f"{inst.name}_xw{i}", ins=[], outs=[]
                        )
                        nop.engine = inst.engine
                        nop.sync_info = mybir.SyncInfo(
                            on_wait=waits[i:i + limit], on_update=[]
                        )
                        il.insert(pos, nop)
                        pos += 1
                        idx += 1
                    inst.sync_info = mybir.SyncInfo(
                        on_wait=waits[:limit], on_update=list(si.on_update)
                    )
                    n_split += 1
                idx += 1
    return n_split


def build_attention_nc() -> bass.Bass:
    nc = bass.Bass("TRN2", target_bir_lowering=False, debug=False)

    xT_d = nc.dram_tensor("xT", [D, S], BF16, kind="ExternalInput").ap()
    wqk_d = nc.dram_tensor("wqk", [D, 8 * DK], BF16, kind="ExternalInput").ap()
    wv_d = nc.dram_tensor("wv", [D, 4 * DK], BF16, kind="ExternalInput").ap()
    wo_d = nc.dram_tensor("wo", [4 * DK, D], BF16, kind="ExternalInput").ap()
    bqk_d = nc.dram_tensor("bqk", [8 * DK, 1], F32, kind="ExternalInput").ap()
    masks_d = nc.dram_tensor("masks", [4, 128, 1024], BF16, kind="ExternalInput").ap()
    y_d = nc.dram_tensor("y", [S, D], BF16, kind="ExternalOutput").ap()

    n_kt = D // 128          # 8 contraction tiles over d
    n_st = S // 128          # 16 seq tiles of 128
    n_qt = S // S_TILE       # 4 q tiles of 512
    AV_LAG = 2               # AV trails exp by this many blocks in the stream

    from contextlib import ExitStack

    with tile.TileContext(nc) as tc, ExitStack() as stack:
        const = stack.enter_context(tc.tile_pool(name="const", bufs=1))
        xpool = stack.enter_context(tc.tile_pool(name="xp", bufs=1))
        kqpool = stack.enter_context(tc.tile_pool(name="kqp", bufs=1))
        vpool = stack.enter_context(tc.tile_pool(name="vp", bufs=1))
        avnpool = stack.enter_context(tc.tile_pool(name="avnp", bufs=1))
        ppool = stack.enter_context(tc.tile_pool(name="pp", bufs=24))
        spool = stack.enter_context(tc.tile_pool(name="sp", bufs=4))
        ypool = stack.enter_context(tc.tile_pool(name="yp", bufs=2))
        avsb = stack.enter_context(tc.tile_pool(name="avsb", bufs=4))
        rec_dr = stack.enter_context(tc.tile_pool(name="rec_dr", bufs=4, space="DRAM"))
        # PSUM: shared 3-deep [128,1024] pool (6 banks) + av pair (2 banks)
        big_ps = stack.enter_context(tc.tile_pool(name="big_ps", bufs=3, space="PSUM"))
        av_ps = stack.enter_context(tc.tile_pool(name="av_ps", bufs=2, space="PSUM"))

        # Prewarm the ACT function table so the 1.3us ACT_TABLE_LOAD happens
        # during the initial DMA waits, not in front of the first kq copy.
        warm = spool.tile([1, 4], F32, tag="warm", name="warm")
        nc.vector.memset(warm, 1.0)
        nc.scalar.activation(warm, warm, mybir.ActivationFunctionType.Exp)

        # --- resident loads (ordered so QKV compute can start early) ----
        # DMA issues spread across sync/vector/gpsimd queues: the ACT engine
        # is kept free (it owns the kq bias-copies), and sync only carries
        # the 8 full-row xT loads (565ns of sequencer time each).
        xT, wqk, wv = [], [], []
        for i in range(n_kt):
            t = xpool.tile([128, S], BF16, tag=f"xT{i}", name=f"xT{i}")
            xT.append(t)
        for i in range(n_kt):
            w1 = const.tile([128, 8 * DK], BF16, tag=f"wqk{i}", name=f"wqk{i}")
            nc.gpsimd.dma_start(out=w1, in_=wqk_d[i * 128:(i + 1) * 128, :])
            wqk.append(w1)
            w2 = const.tile([128, 4 * DK], BF16, tag=f"wv{i}", name=f"wv{i}")
            nc.gpsimd.dma_start(out=w2, in_=wv_d[i * 128:(i + 1) * 128, :])
            wv.append(w2)
            nc.sync.dma_start(out=xT[i], in_=xT_d[i * 128:(i + 1) * 128, :])
        bqk = []
        for i in range(4):
            t = const.tile([128, 1], F32, tag=f"bqk{i}", name=f"bqk{i}")
            nc.gpsimd.dma_start(out=t, in_=bqk_d[i * 128:(i + 1) * 128, :])
            bqk.append(t)
        masks = []
        for i in range(4):
            t = const.tile([128, 1024], BF16, tag=f"mask{i}", name=f"mask{i}")
            nc.gpsimd.dma_start(out=t, in_=masks_d[i])
            masks.append(t)
        wo = []
        for i in range(2):
            t = const.tile([128, D], BF16, tag=f"wo{i}", name=f"wo{i}")
            nc.gpsimd.dma_start(out=t, in_=wo_d[i * 128:(i + 1) * 128, :])
            wo.append(t)

        # kq[m][f, s]: m=0 -> k heads(0,1); 1 -> k heads(2,3); 2 -> q(0,1); 3 -> q(2,3)
        kq = [kqpool.tile([128, S], BF16, tag=f"kq{m}", name=f"kq{m}") for m in range(4)]
        # v_sb[st][128, 4*65]: per head h: cols [h*65, h*65+64) = v, col h*65+64 = 1.0
        v_sb = [vpool.tile([128, HPC * (DK + 1)], BF16, tag=f"v{st}", name=f"v{st}")
                for st in range(n_st)]
        # avn[f2][f, s]: f2=0 -> heads (0,1); f2=1 -> heads (2,3)
        avn = [avnpool.tile([128, S], BF16, tag=f"avn{f2}", name=f"avn{f2}")
               for f2 in range(2)]

        def emit_kq(m, sq):
            ps = big_ps.tile([128, S_TILE], F32, tag="bigps", name="qkps")
            for kt in range(n_kt):
                nc.tensor.matmul(
                    ps,
                    lhsT=wqk[kt][:, m * 128:(m + 1) * 128],
                    rhs=xT[kt][:, sq * S_TILE:(sq + 1) * S_TILE],
                    start=(kt == 0),
                    stop=(kt == n_kt - 1),
                )
            # psum -> sbuf with per-feature bias, on ACT (idle in QKV phase)
            nc.scalar.activation(
                kq[m][:, sq * S_TILE:(sq + 1) * S_TILE], ps,
                mybir.ActivationFunctionType.Identity, bias=bqk[m],
            )

        def emit_v(st):
            ps = big_ps.tile([128, HPC * DK], F32, tag="bigps", name="vps")
            for kt in range(n_kt):
                nc.tensor.matmul(
                    ps,
                    lhsT=xT[kt][:, st * 128:(st + 1) * 128],
                    rhs=wv[kt],
                    start=(kt == 0),
                    stop=(kt == n_kt - 1),
                )
            nc.gpsimd.memset(v_sb[st], 1.0)
            for h in range(HPC):
                nc.vector.tensor_copy(
                    out=v_sb[st][:, h * (DK + 1):h * (DK + 1) + DK],
                    in_=ps[:, h * DK:(h + 1) * DK],
                )

        def emit_wo(st):
            yp = big_ps.tile([128, D], F32, tag="bigps", name="yps")
            for oh in range(2):
                for f2 in range(2):
                    nc.tensor.matmul(
                        yp[:, oh * 512:(oh + 1) * 512],
                        lhsT=avn[f2][:, st * 128:(st + 1) * 128],
                        rhs=wo[f2][:, oh * 512:(oh + 1) * 512],
                        start=(f2 == 0),
                        stop=(f2 == 1),
                    )
            y_sb = ypool.tile([128, D], BF16, tag="ysb", name="ysb")
            nc.vector.tensor_copy(out=y_sb, in_=yp)
            nc.sync.dma_start(out=y_d[st * 128:(st + 1) * 128, :], in_=y_sb)

        def qkv_round(sq):
            return [
                lambda m=m, sq=sq: emit_kq(m, sq) for m in (0, 2, 1, 3)
            ] + [lambda st=st: emit_v(st) for st in range(4 * sq, 4 * sq + 4)]

        def attention_tile(t, jobs):
            """Emit attention for q-tile t, interleaving `jobs` (QKV groups of
            the next round, deferred w_o tiles) into the stream. AV matmuls
            trail their exp by AV_LAG blocks so the in-order PE stream never
            parks on an unfinished exp."""
            nblk = 4 * t + 4
            stride = max(1, (2 * nblk) // max(1, len(jobs)))
            s = 0
            for hp in range(2):
                kt2 = kq[hp]
                qt2 = kq[2 + hp]
                av_t = [av_ps.tile([128, S_TILE], F32, tag="avps", name="avps")
                        for _ in range(2)]
                pend = []

                def emit_av(blk, p):
                    for i in range(2):
                        h = 2 * hp + i
                        nc.tensor.matmul(
                            av_t[i][0:DK + 1, :],
                            lhsT=v_sb[blk][:, h * (DK + 1):(h + 1) * (DK + 1)],
                            rhs=p[:, i * S_TILE:(i + 1) * S_TILE],
                            start=(blk == 0),
                            stop=(blk == nblk - 1),
                        )

                for blk in range(nblk):
                    if jobs and s % stride == 0:
                        jobs.pop(0)()
                    s += 1
                    sc = big_ps.tile([128, 2 * S_TILE], F32, tag="bigps", name="scps")
                    for i in range(2):  # head A / head B, row-tiled pair
                        nc.tensor.matmul(
                            sc[:, i * S_TILE:(i + 1) * S_TILE],
                            lhsT=kt2[i * 64:(i + 1) * 64, blk * K_BLK:(blk + 1) * K_BLK],
                            rhs=qt2[i * 64:(i + 1) * 64, t * S_TILE:(t + 1) * S_TILE],
                            start=True,
                            stop=True,
                            tile_position=(i * 64, 0),
                        )
                    p = ppool.tile([128, 2 * S_TILE], BF16, tag="p", name="p")
                    nc.scalar.activation(p, sc, mybir.ActivationFunctionType.Exp,
                                         scale=0.125)
                    dd = blk - 4 * t
                    if dd >= 0:       # diagonal block: apply causal 0/1 mask
                        nc.vector.tensor_mul(p, p, masks[dd])
                    pend.append((blk, p))
                    if len(pend) > AV_LAG:
                        emit_av(*pend.pop(0))
                while pend:
                    if jobs and s % stride == 0:
                        jobs.pop(0)()
                    s += 1
                    emit_av(*pend.pop(0))
                # move av (+denominator row) off PSUM right away
                av_c = []
                for i in range(2):
                    c = avsb.tile([DK + 1, S_TILE], F32, tag="avc", name="avc")
                    nc.vector.tensor_copy(out=c, in_=av_t[i][0:DK + 1, :])
                    av_c.append(c)
                # normalize: DVE reciprocal cost scales with free-dim size, so
                # reshape the [1,512] denominator row to [128,4] with a direct
                # SBUF->SBUF DMA, reciprocal there, then broadcast back down
                # the 64 dk partitions via a DRAM bounce (SBUF APs cannot
                # express a 0-stride partition read). Head i=1 goes first so
                # its cross-partition move (rows 0-63 -> 64-127) overlaps
                # head i=0's normalize chain.
                for i in (1, 0):
                    den_d = rec_dr.tile([S_TILE], F32, tag="dend", name="dend")
                    nc.gpsimd.dma_start(out=den_d, in_=av_c[i][DK:DK + 1, :])
                    den2 = spool.tile([128, 4], F32, tag="den2", name="den2")
                    nc.gpsimd.dma_start(
                        out=den2, in_=den_d.rearrange("(p f) -> p f", p=128)
                    )
                    nc.vector.reciprocal(den2, den2)
                    rec_d = rec_dr.tile([S_TILE], F32, tag="recd", name="recd")
                    nc.gpsimd.dma_start(
                        out=rec_d.rearrange("(p f) -> p f", p=128), in_=den2
                    )
                    bc = spool.tile([DK, S_TILE], F32, tag="bc", name="bc")
                    nc.gpsimd.dma_start(
                        out=bc,
                        in_=rec_d.rearrange("(a f) -> a f", a=1)
                                 .partition_broadcast(DK),
                    )
                    if i == 0:
                        dst = avn[hp][0:DK, t * S_TILE:(t + 1) * S_TILE]
                        nc.gpsimd.tensor_mul(dst, av_c[i][0:DK, :], bc)
                    else:
                        tmp = spool.tile([DK, S_TILE], BF16, tag="avtmp", name="avtmp")
                        nc.gpsimd.tensor_mul(tmp, av_c[i][0:DK, :], bc)
                        nc.gpsimd.dma_start(
                            out=avn[hp][64:128, t * S_TILE:(t + 1) * S_TILE],
                            in_=tmp,
                        )
            while jobs:
                jobs.pop(0)()

        for job in qkv_round(0):
            job()
        for t in range(n_qt):
            if t + 1 < n_qt:
                jobs = list(qkv_round(t + 1))
            else:
                # last tile: no QKV rounds left — fill PE gaps with the
                # deferred w_o projections of tiles 0..2
                jobs = [lambda st=st: emit_wo(st) for st in range(12)]
            attention_tile(t, jobs)
        for st in range(12, n_st):
            emit_wo(st)

    n = _split_excess_waits(nc)
    salt = mybir.InstNoOp(name=f"salt_{_CFG_SALT}", ins=[], outs=[])
    salt.engine = mybir.EngineType.SP
    nc.m.functions[0].blocks[0].instructions.insert(0, salt)
    return nc


_CACHED_NC = None


def _get_nc():
    global _CACHED_NC
    if _CACHED_NC is None:
        _CACHED_NC = build_attention_nc()
    return _CACHED_NC


def _prep_core_inputs(x, mask, w_qkv_w, w_qkv_b, w_o_w, w_o_b, core):
    b = core // 4
    hg = core % 4
    heads = [hg * HPC + h for h in range(HPC)]

    xT = np.ascontiguousarray(x[b].T).astype(NP_BF16)

    def rows(sec, h):  # q=0, k=1, v=2
        base = sec * D + h * DK
        return slice(base, base + DK)

    wqk_rows = np.concatenate(
        [w_qkv_w[rows(1, h)] for h in heads] + [w_qkv_w[rows(0, h)] for h in heads],
        axis=0,
    )  # [512, 1024]
    wqk = np.ascontiguousarray(wqk_rows.T).astype(NP_BF16)

    wv_rows = np.concatenate([w_qkv_w[rows(2, h)] for h in heads], axis=0)
    wv = np.ascontiguousarray(wv_rows.T).astype(NP_BF16)

    wo = np.ascontiguousarray(
        w_o_w[:, hg * HPC * DK:(hg + 1) * HPC * DK].T
    ).astype(NP_BF16)

    bqk = np.concatenate(
        [w_qkv_b[rows(1, h)] for h in heads] + [w_qkv_b[rows(0, h)] for h in heads]
    ).astype(np.float32)[:, None]

    # Diagonal-block mask patterns from the provided mask tensor.
    m2d = np.asarray(mask[0, 0])
    q0 = S - S_TILE
    pats = []
    for dd in range(4):
        k0 = q0 + dd * K_BLK
        pat = m2d[q0:q0 + S_TILE, k0:k0 + K_BLK].T.astype(np.float32)  # [128, 512]
        pats.append(np.concatenate([pat, pat], axis=1))               # [128, 1024]
    masks_np = np.stack(pats).astype(NP_BF16)

    return {
        "xT": xT, "wqk": wqk, "wv": wv, "wo": wo,
        "bqk": bqk, "masks": masks_np,
    }


def kernel(x, mask, w_qkv_w, w_qkv_b, w_o_w, w_o_b, _profile=False):
    x = np.asarray(x, np.float32)
    w_qkv_w = np.asarray(w_qkv_w, np.float32)
    w_qkv_b = np.asarray(w_qkv_b, np.float32)
    w_o_w = np.asarray(w_o_w, np.float32)
    w_o_b = np.asarray(w_o_b, np.float32)

    nc = _get_nc()
    in_maps = [
        _prep_core_inputs(x, mask, w_qkv_w, w_qkv_b, w_o_w, w_o_b, c)
        for c in range(N_CORES)
    ]
    res = run_bass_kernel_spmd(
        nc, in_maps, core_ids=list(range(N_CORES)), trace=_profile
    )
    y = np.zeros((B, S, D), np.float32)
    for c in range(N_CORES):
        y[c // 4] += np.asarray(res.results[c]["y"]).astype(np.float32)
    # bias: w_o bias plus the v-bias folded through the softmax average
    y += (w_o_b + w_o_w @ w_qkv_b[2 * D:3 * D])[None, None, :]
    if _profile:
        return y, res
    return y


# revision 22
# speedup vs baseline: 1.1998x; 1.0122x over previous
"""Causal multi-head attention (B=2, S=2048, D=1024, H=16) on 8 TRN2 NeuronCores.

Sharding: core c handles batch b = c//4 and heads [4*(c%4), 4*(c%4)+4).
Each core computes its 4 heads' attention plus the partial w_o projection;
the host sums the 4 partials per batch (the "all-reduce after w_o") and
adds the output bias plus the v-bias term folded through the softmax
(sum_k p_k (v+bv) / den = av/den + bv, and bv^T w_o is a constant row).

Compute dtype: bf16 matmul inputs with fp32 PSUM accumulation; y partials
written back in bf16 (summed in fp32 on host).

Layouts (per core, host-prepared):
  xT    [1024, 2048] bf16  x[b].T                    (d on partitions)
  wqk   [1024, 512]  bf16  cols = [k_h0..k_h3 | q_h0..q_h3] (64 each)
  wv    [1024, 256]  bf16  cols = [v_h0..v_h3]
  wo    [256, 1024]  bf16  w_o[:, head_cols].T
  bqk   [512, 1]     f32   per-feature bias, same col order as wqk
  masks [4, 128, 1024] bf16  causal 0/1 patterns for diagonal blocks

In-kernel dataflow (per core):
  qkvT = wqk.T @ x.T  -> k/q in [feature, seq] layout, paired heads per tile
  v    = x @ wv       -> natural [seq, feature] + a ones column per head
  scores_T[k,q] = k_h.T(dk x 128) @ q_h(dk x 512)   (two heads row-tiled)
  p = exp(scores/8)   (ACT, psum->sbuf, bf16) ; diag blocks masked via DVE
  av_T[dk+1, q] = [v_h | 1].T @ p                   (row 64 = softmax denom)
  avn = av * (1/denom broadcast)                    (DVE mul, -> bf16)
  y[s, o] = avn.T @ wo                              (partial, bf16 to HBM)

PSUM budget (8 banks): one shared 3-deep pool of [128,1024] tiles (6 banks)
serves scores, kq/v projection and w_o psum; av accumulators 2x[128,512].
Score-buffer depth 3 decouples the scores->exp->scores recycle so the PE
streams while ACT exps; w_o jobs are deferred to the last (largest)
attention tile where no QKV rounds remain to fill PE gaps.
"""

import numpy as np
import ml_dtypes

import concourse.bass as bass
import concourse.mybir as mybir
import concourse.tile as tile
from concourse.bass_utils import run_bass_kernel_spmd
from concourse.vector_clock import ScopedClock

BF16 = mybir.dt.bfloat16
F32 = mybir.dt.float32
NP_BF16 = ml_dtypes.bfloat16

B, S, D = 2, 2048, 1024
H, DK = 16, 64
HPC = 4            # heads per core
N_CORES = 8
S_TILE = 512       # q tile width (f32 psum bank)
K_BLK = 128        # k block (partition dim of transposed scores)


# ---------------------------------------------------------------------------
# Workaround: this walrus build rejects >1 sem-wait on the TileContext exit
# Drain. Redistribute the global-clock waits onto single-wait sync NOPs.
# ---------------------------------------------------------------------------
def _patched_drain_and_barrier(self, tick_clock, wait_clock):
    probe = self.nc.sync.nop()
    wait_clock.add_sem_waits(probe.ins, ScopedClock({None: tick_clock.global_clock}))
    si = probe.ins.sync_info
    waits = list(si.on_wait)
    probe.ins.sync_info = mybir.SyncInfo(on_wait=waits[:1], on_update=list(si.on_update))
    for w in waits[1:]:
        nop = self.nc.sync.nop()
        nop.ins.sync_info = mybir.SyncInfo(on_wait=[w], on_update=[])
    self.nc.sync.drain()

    self.nc.all_engine_barrier()
    assert self.sems is not None
    popped = self.nc._tile_sem_poison_stack.pop()
    assert popped is self._sem_poison
    self.nc.clear_and_free_semaphores(list(self.sems.allocated().values()))
    self.nc.all_engine_barrier()


tile.TileContext._drain_and_barrier = _patched_drain_and_barrier

_CFG_SALT = "cfg-v2"

_WAIT_LIMIT = 1


def _split_excess_waits(nc: bass.Bass, limit: int = _WAIT_LIMIT):
    """Walrus (this build) rejects instructions carrying more than a couple of
    sem waits. Move excess waits onto same-engine NOPs inserted just before."""
    n_split = 0
    for f in nc.m.functions:
        for bb in f.blocks:
            il = bb.instructions
            idx = 0
            while idx < len(il):
                inst = il[idx]
                si = inst.sync_info
                if si is not None and len(si.on_wait) > limit:
                    waits = list(si.on_wait)
                    pos = idx
                    for i in range(limit, len(waits), limit):
                        nop = mybir.InstNoOp(
                            name=f"{inst.name}_xw{i}", ins=[], outs=[]
                        )
                        nop.engine = inst.engine
                        nop.sync_info = mybir.SyncInfo(
                            on_wait=waits[i:i + limit], on_update=[]
                        )
                        il.insert(pos, nop)
                        pos += 1
                        idx += 1
                    inst.sync_info = mybir.SyncInfo(
                        on_wait=waits[:limit], on_update=list(si.on_update)
                    )
                    n_split += 1
                idx += 1
    return n_split


def build_attention_nc() -> bass.Bass:
    nc = bass.Bass("TRN2", target_bir_lowering=False, debug=False)

    xT_d = nc.dram_tensor("xT", [D, S], BF16, kind="ExternalInput").ap()
    wqk_d = nc.dram_tensor("wqk", [D, 8 * DK], BF16, kind="ExternalInput").ap()
    wv_d = nc.dram_tensor("wv", [D, 4 * DK], BF16, kind="ExternalInput").ap()
    wo_d = nc.dram_tensor("wo", [4 * DK, D], BF16, kind="ExternalInput").ap()
    bqk_d = nc.dram_tensor("bqk", [8 * DK, 1], F32, kind="ExternalInput").ap()
    masks_d = nc.dram_tensor("masks", [4, 128, 1024], BF16, kind="ExternalInput").ap()
    y_d = nc.dram_tensor("y", [S, D], BF16, kind="ExternalOutput").ap()

    n_kt = D // 128          # 8 contraction tiles over d
    n_st = S // 128          # 16 seq tiles of 128
    n_qt = S // S_TILE       # 4 q tiles of 512
    AV_LAG = 2               # AV trails exp by this many blocks in the stream

    from contextlib import ExitStack

    with tile.TileContext(nc) as tc, ExitStack() as stack:
        const = stack.enter_context(tc.tile_pool(name="const", bufs=1))
        xpool = stack.enter_context(tc.tile_pool(name="xp", bufs=1))
        kqpool = stack.enter_context(tc.tile_pool(name="kqp", bufs=1))
        vpool = stack.enter_context(tc.tile_pool(name="vp", bufs=1))
        avnpool = stack.enter_context(tc.tile_pool(name="avnp", bufs=1))
        ppool = stack.enter_context(tc.tile_pool(name="pp", bufs=24))
        spool = stack.enter_context(tc.tile_pool(name="sp", bufs=4))
        ypool = stack.enter_context(tc.tile_pool(name="yp", bufs=2))
        avsb = stack.enter_context(tc.tile_pool(name="avsb", bufs=2))
        rec_dr = stack.enter_context(tc.tile_pool(name="rec_dr", bufs=4, space="DRAM"))
        # PSUM: shared 3-deep [128,1024] pool (6 banks) + av pair (2 banks)
        big_ps = stack.enter_context(tc.tile_pool(name="big_ps", bufs=3, space="PSUM"))
        av_ps = stack.enter_context(tc.tile_pool(name="av_ps", bufs=2, space="PSUM"))

        # Prewarm the ACT function table so the 1.3us ACT_TABLE_LOAD happens
        # during the initial DMA waits, not in front of the first kq copy.
        warm = spool.tile([1, 4], F32, tag="warm", name="warm")
        nc.vector.memset(warm, 1.0)
        nc.scalar.activation(warm, warm, mybir.ActivationFunctionType.Exp)

        # --- resident loads (ordered so QKV compute can start early) ----
        # DMA issues spread across sync/vector/gpsimd queues: the ACT engine
        # is kept free (it owns the kq bias-copies), and sync only carries
        # the 8 full-row xT loads (565ns of sequencer time each).
        xT, wqk, wv = [], [], []
        for i in range(n_kt):
            t = xpool.tile([128, S], BF16, tag=f"xT{i}", name=f"xT{i}")
            xT.append(t)
        for i in range(n_kt):
            w1 = const.tile([128, 8 * DK], BF16, tag=f"wqk{i}", name=f"wqk{i}")
            nc.gpsimd.dma_start(out=w1, in_=wqk_d[i * 128:(i + 1) * 128, :])
            wqk.append(w1)
            w2 = const.tile([128, 4 * DK], BF16, tag=f"wv{i}", name=f"wv{i}")
            nc.gpsimd.dma_start(out=w2, in_=wv_d[i * 128:(i + 1) * 128, :])
            wv.append(w2)
            nc.sync.dma_start(
                out=xT[i][:, 0:2 * S_TILE],
                in_=xT_d[i * 128:(i + 1) * 128, 0:2 * S_TILE],
            )
        for i in range(n_kt):
            nc.sync.dma_start(
                out=xT[i][:, 2 * S_TILE:S],
                in_=xT_d[i * 128:(i + 1) * 128, 2 * S_TILE:S],
            )
        bqk = []
        for i in range(4):
            t = const.tile([128, 1], F32, tag=f"bqk{i}", name=f"bqk{i}")
            nc.gpsimd.dma_start(out=t, in_=bqk_d[i * 128:(i + 1) * 128, :])
            bqk.append(t)
        masks = []
        for i in range(4):
            t = const.tile([128, 1024], BF16, tag=f"mask{i}", name=f"mask{i}")
            nc.gpsimd.dma_start(out=t, in_=masks_d[i])
            masks.append(t)
        wo = []
        for i in range(2):
            t = const.tile([128, D], BF16, tag=f"wo{i}", name=f"wo{i}")
            nc.gpsimd.dma_start(out=t, in_=wo_d[i * 128:(i + 1) * 128, :])
            wo.append(t)

        # kq[m][f, s]: m=0 -> k heads(0,1); 1 -> k heads(2,3); 2 -> q(0,1); 3 -> q(2,3)
        kq = [kqpool.tile([128, S], BF16, tag=f"kq{m}", name=f"kq{m}") for m in range(4)]
        # v_sb[st][128, 4*65]: per head h: cols [h*65, h*65+64) = v, col h*65+64 = 1.0
        v_sb = [vpool.tile([128, HPC * (DK + 1)], BF16, tag=f"v{st}", name=f"v{st}")
                for st in range(n_st)]
        # avn[f2][f, s]: f2=0 -> heads (0,1); f2=1 -> heads (2,3)
        avn = [avnpool.tile([128, S], BF16, tag=f"avn{f2}", name=f"avn{f2}")
               for f2 in range(2)]

        def emit_kq(m, sq):
            ps = big_ps.tile([128, S_TILE], F32, tag="bigps", name="qkps")
            for kt in range(n_kt):
                nc.tensor.matmul(
                    ps,
                    lhsT=wqk[kt][:, m * 128:(m + 1) * 128],
                    rhs=xT[kt][:, sq * S_TILE:(sq + 1) * S_TILE],
                    start=(kt == 0),
                    stop=(kt == n_kt - 1),
                )
            # psum -> sbuf with per-feature bias, on ACT (idle in QKV phase)
            nc.scalar.activation(
                kq[m][:, sq * S_TILE:(sq + 1) * S_TILE], ps,
                mybir.ActivationFunctionType.Identity, bias=bqk[m],
            )

        def emit_v(st):
            ps = big_ps.tile([128, HPC * DK], F32, tag="bigps", name="vps")
            for kt in range(n_kt):
                nc.tensor.matmul(
                    ps,
                    lhsT=xT[kt][:, st * 128:(st + 1) * 128],
                    rhs=wv[kt],
                    start=(kt == 0),
                    stop=(kt == n_kt - 1),
                )
            nc.gpsimd.memset(v_sb[st], 1.0)
            for h in range(HPC):
                nc.vector.tensor_copy(
                    out=v_sb[st][:, h * (DK + 1):h * (DK + 1) + DK],
                    in_=ps[:, h * DK:(h + 1) * DK],
                )

        def emit_wo(st):
            yp = big_ps.tile([128, D], F32, tag="bigps", name="yps")
            for oh in range(2):
                for f2 in range(2):
                    nc.tensor.matmul(
                        yp[:, oh * 512:(oh + 1) * 512],
                        lhsT=avn[f2][:, st * 128:(st + 1) * 128],
                        rhs=wo[f2][:, oh * 512:(oh + 1) * 512],
                        start=(f2 == 0),
                        stop=(f2 == 1),
                    )
            y_sb = ypool.tile([128, D], BF16, tag="ysb", name="ysb")
            nc.vector.tensor_copy(out=y_sb, in_=yp)
            nc.sync.dma_start(out=y_d[st * 128:(st + 1) * 128, :], in_=y_sb)

        def qkv_round(sq):
            return [
                lambda m=m, sq=sq: emit_kq(m, sq) for m in (0, 2, 1, 3)
            ] + [lambda st=st: emit_v(st) for st in range(4 * sq, 4 * sq + 4)]

        def attention_tile(t, jobs):
            """Emit attention for q-tile t, interleaving `jobs` (QKV groups of
            the next round, deferred w_o tiles) into the stream. AV matmuls
            trail their exp by AV_LAG blocks so the in-order PE stream never
            parks on an unfinished exp."""
            nblk = 4 * t + 4
            stride = max(1, (2 * nblk) // max(1, len(jobs)))
            s = 0
            for hp in range(2):
                kt2 = kq[hp]
                qt2 = kq[2 + hp]
                av_t = [av_ps.tile([128, S_TILE], F32, tag="avps", name="avps")
                        for _ in range(2)]
                pend = []

                def emit_av(blk, p):
                    for i in range(2):
                        h = 2 * hp + i
                        nc.tensor.matmul(
                            av_t[i][0:DK + 1, :],
                            lhsT=v_sb[blk][:, h * (DK + 1):(h + 1) * (DK + 1)],
                            rhs=p[:, i * S_TILE:(i + 1) * S_TILE],
                            start=(blk == 0),
                            stop=(blk == nblk - 1),
                        )

                for blk in range(nblk):
                    if jobs and s % stride == 0:
                        jobs.pop(0)()
                    s += 1
                    sc = big_ps.tile([128, 2 * S_TILE], F32, tag="bigps", name="scps")
                    for i in range(2):  # head A / head B, row-tiled pair
                        nc.tensor.matmul(
                            sc[:, i * S_TILE:(i + 1) * S_TILE],
                            lhsT=kt2[i * 64:(i + 1) * 64, blk * K_BLK:(blk + 1) * K_BLK],
                            rhs=qt2[i * 64:(i + 1) * 64, t * S_TILE:(t + 1) * S_TILE],
                            start=True,
                            stop=True,
                            tile_position=(i * 64, 0),
                        )
                    p = ppool.tile([128, 2 * S_TILE], BF16, tag="p", name="p")
                    nc.scalar.activation(p, sc, mybir.ActivationFunctionType.Exp,
                                         scale=0.125)
                    dd = blk - 4 * t
                    if dd >= 0:       # diagonal block: apply causal 0/1 mask
                        nc.vector.tensor_mul(p, p, masks[dd])
                    pend.append((blk, p))
                    if len(pend) > AV_LAG:
                        emit_av(*pend.pop(0))
                while pend:
                    if jobs and s % stride == 0:
                        jobs.pop(0)()
                    s += 1
                    emit_av(*pend.pop(0))
                # move av (+denominator row) off PSUM right away, both heads
                # side-by-side in one tile so the denominator chain below
                # handles the pair with a single 4-hop bounce.
                av_c = avsb.tile([DK + 1, 2 * S_TILE], F32, tag="avc", name="avc")
                for i in range(2):
                    nc.vector.tensor_copy(
                        out=av_c[:, i * S_TILE:(i + 1) * S_TILE],
                        in_=av_t[i][0:DK + 1, :],
                    )
                # normalize: DVE reciprocal cost scales with per-partition
                # free-dim size, so re-partition the [1,1024] denominator row
                # pair to [128,8] via a DRAM hop (SBUF APs cannot cross
                # partitions), reciprocal there, then broadcast back down the
                # 64 dk partitions via a second DRAM hop.
                den_d = rec_dr.tile([2 * S_TILE], F32, tag="dend", name="dend")
                nc.gpsimd.dma_start(out=den_d, in_=av_c[DK:DK + 1, :])
                den2 = spool.tile([128, 8], F32, tag="den2", name="den2")
                nc.gpsimd.dma_start(
                    out=den2, in_=den_d.rearrange("(p f) -> p f", p=128)
                )
                nc.vector.reciprocal(den2, den2)
                rec_d = rec_dr.tile([2 * S_TILE], F32, tag="recd", name="recd")
                nc.gpsimd.dma_start(
                    out=rec_d.rearrange("(p f) -> p f", p=128), in_=den2
                )
                bc = spool.tile([DK, 2 * S_TILE], F32, tag="bc", name="bc")
                nc.gpsimd.dma_start(
                    out=bc,
                    in_=rec_d.rearrange("(a f) -> a f", a=1).partition_broadcast(DK),
                )
                # head i=1 first: its cross-partition move (rows 0-63 ->
                # 64-127) overlaps head i=0's multiply.
                tmp = spool.tile([DK, S_TILE], BF16, tag="avtmp", name="avtmp")
                nc.gpsimd.tensor_mul(
                    tmp, av_c[0:DK, S_TILE:2 * S_TILE], bc[:, S_TILE:2 * S_TILE]
                )
                nc.gpsimd.dma_start(
                    out=avn[hp][64:128, t * S_TILE:(t + 1) * S_TILE], in_=tmp
                )
                dst = avn[hp][0:DK, t * S_TILE:(t + 1) * S_TILE]
                nc.vector.tensor_mul(dst, av_c[0:DK, 0:S_TILE], bc[:, 0:S_TILE])
            while jobs:
                jobs.pop(0)()

        def emit_wo_tail(st):
            # Tail variant: big_ps slots are clogged by the final exp stream,
            # av_ps is free once the last normalize copied out. Two [128,512]
            # halves rotate through the 2 av banks.
            y_sb = ypool.tile([128, D], BF16, tag="ysb", name="ysb")
            for oh in range(2):
                yp = av_ps.tile([128, S_TILE], F32, tag="avps", name="yph")
                for f2 in range(2):
                    nc.tensor.matmul(
                        yp,
                        lhsT=avn[f2][:, st * 128:(st + 1) * 128],
                        rhs=wo[f2][:, oh * 512:(oh + 1) * 512],
                        start=(f2 == 0),
                        stop=(f2 == 1),
                    )
                nc.vector.tensor_copy(out=y_sb[:, oh * 512:(oh + 1) * 512], in_=yp)
            nc.sync.dma_start(out=y_d[st * 128:(st + 1) * 128, :], in_=y_sb)

        for job in qkv_round(0):
            job()
        for t in range(n_qt):
            if t + 1 < n_qt:
                jobs = list(qkv_round(t + 1))
            else:
                # last tile: no QKV rounds left — fill PE gaps with the
                # deferred w_o projections of tiles 0..2
                jobs = [lambda st=st: emit_wo(st) for st in range(12)]
            attention_tile(t, jobs)
        for st in range(12, n_st):
            emit_wo_tail(st)

    n = _split_excess_waits(nc)
    salt = mybir.InstNoOp(name=f"salt_{_CFG_SALT}", ins=[], outs=[])
    salt.engine = mybir.EngineType.SP
    nc.m.functions[0].blocks[0].instructions.insert(0, salt)
    return nc


_CACHED_NC = None


def _get_nc():
    global _CACHED_NC
    if _CACHED_NC is None:
        _CACHED_NC = build_attention_nc()
    return _CACHED_NC


def _prep_core_inputs(x, mask, w_qkv_w, w_qkv_b, w_o_w, w_o_b, core):
    b = core // 4
    hg = core % 4
    heads = [hg * HPC + h for h in range(HPC)]

    xT = np.ascontiguousarray(x[b].T).astype(NP_BF16)

    def rows(sec, h):  # q=0, k=1, v=2
        base = sec * D + h * DK
        return slice(base, base + DK)

    wqk_rows = np.concatenate(
        [w_qkv_w[rows(1, h)] for h in heads] + [w_qkv_w[rows(0, h)] for h in heads],
        axis=0,
    )  # [512, 1024]
    wqk = np.ascontiguousarray(wqk_rows.T).astype(NP_BF16)

    wv_rows = np.concatenate([w_qkv_w[rows(2, h)] for h in heads], axis=0)
    wv = np.ascontiguousarray(wv_rows.T).astype(NP_BF16)

    wo = np.ascontiguousarray(
        w_o_w[:, hg * HPC * DK:(hg + 1) * HPC * DK].T
    ).astype(NP_BF16)

    bqk = np.concatenate(
        [w_qkv_b[rows(1, h)] for h in heads] + [w_qkv_b[rows(0, h)] for h in heads]
    ).astype(np.float32)[:, None]

    # Diagonal-block mask patterns from the provided mask tensor.
    m2d = np.asarray(mask[0, 0])
    q0 = S - S_TILE
    pats = []
    for dd in range(4):
        k0 = q0 + dd * K_BLK
        pat = m2d[q0:q0 + S_TILE, k0:k0 + K_BLK].T.astype(np.float32)  # [128, 512]
        pats.append(np.concatenate([pat, pat], axis=1))               # [128, 1024]
    masks_np = np.stack(pats).astype(NP_BF16)

    return {
        "xT": xT, "wqk": wqk, "wv": wv, "wo": wo,
        "bqk": bqk, "masks": masks_np,
    }


def kernel(x, mask, w_qkv_w, w_qkv_b, w_o_w, w_o_b, _profile=False):
    x = np.asarray(x, np.float32)
    w_qkv_w = np.asarray(w_qkv_w, np.float32)
    w_qkv_b = np.asarray(w_qkv_b, np.float32)
    w_o_w = np.asarray(w_o_w, np.float32)
    w_o_b = np.asarray(w_o_b, np.float32)

    nc = _get_nc()
    in_maps = [
        _prep_core_inputs(x, mask, w_qkv_w, w_qkv_b, w_o_w, w_o_b, c)
        for c in range(N_CORES)
    ]
    res = run_bass_kernel_spmd(
        nc, in_maps, core_ids=list(range(N_CORES)), trace=_profile
    )
    y = np.zeros((B, S, D), np.float32)
    for c in range(N_CORES):
        y[c // 4] += np.asarray(res.results[c]["y"]).astype(np.float32)
    # bias: w_o bias plus the v-bias folded through the softmax average
    y += (w_o_b + w_o_w @ w_qkv_b[2 * D:3 * D])[None, None, :]
    if _profile:
        return y, res
    return y
